# revision 4
# baseline (speedup 1.0000x reference)
"""Trainium2 Bass kernel for nn_MultiHeadAttention (B=4, C=1024, T=1024, H=16).

Sharding: 8 cores = (batch b in 0..3) x (head-group g in 0..1), 8 heads per
group. Each core computes q/k/v projections for its group's 512 channels,
rope, attention, and a partial O-projection Wo[:, group] @ att. The host sums
the two partials per batch (bias bo is supplied only to g=0 cores).

Layout strategy (everything stays in natural [channel, t] / [s, t] layouts,
zero on-device transposes; weights are pre-transposed on the host):
  - scores computed transposed: scoresT[s, t] = k[d, s].T @ q[d, t] per head,
    two heads packed per 128-partition tile via PE row-tiling.
  - softmax: no max-subtraction (scores are O(5), mask is all-ones so it is
    skipped), exp on ScalarE straight from PSUM, denominator = extra ones
    column in v^T so the PV matmul emits it for free; normalization via
    reciprocal + gpsimd partition_broadcast.
  - rope folded into the projections: q_rope = C.*q + S.*(P q) where P is a
    fixed signed channel permutation; (P Wq) is computed on the host, the
    elementwise part runs on VectorE reading both projection PSUMs directly.
  - all matmuls run in float32r (full PE rate, ~TF32 precision, fp32 storage).
  - biases are added via K=1 rank-1 matmuls (bias row x ones row) inside each
    accumulation group.
"""
import sys
import time

sys.path.insert(0, '/opt/trn_rl_repo')

import numpy as np

B = 4
C = 1024
T = 1024
H = 16
HD = C // H            # 64
D_ROPE = HD // 2       # 32
HALF = D_ROPE // 2     # 16
GROUPS = 2
NCORES = 8
NH = H // GROUPS       # 8 heads per group
CHG = NH * HD          # 512 channels per group
KT = C // 128          # 8 input-channel k-tiles
ST = T // 128          # 8 s-tiles
TC = 512
NT = T // TC           # 2 t-chunks
MT = CHG // 128        # 4 projection m-tiles per group
OMT = C // 128         # 8 output m-tiles
NPAIR = NH // 2        # 4 head-pairs (2 heads packed per 128-tile)
SCALE = 0.125          # 1/sqrt(HD)

_cache = {}


def _rope_tables():
    theta = 1.0 / (10000.0 ** (np.arange(HALF, dtype=np.float64) * 2.0 / D_ROPE))
    ang = np.arange(T, dtype=np.float64)[:, None] * theta[None, :]   # [T, HALF]
    cos = np.concatenate([np.cos(ang), np.cos(ang)], axis=1)         # [T, D_ROPE]
    sin = np.concatenate([np.sin(ang), np.sin(ang)], axis=1)
    return cos.astype(np.float32), sin.astype(np.float32)


def _cs_tiles():
    """C,S tables in [128 ch, T] layout; the 2-head (64-row) pattern repeats,
    so one 128-row tile serves every projection m-tile."""
    cos, sin = _rope_tables()
    Ct = np.ones((128, T), dtype=np.float32)
    St = np.zeros((128, T), dtype=np.float32)
    for h in range(2):
        o = h * HD
        Ct[o:o + D_ROPE, :] = cos.T
        St[o:o + D_ROPE, :] = sin.T
    return Ct, St


def _perm_rows(W):
    """Apply the rope 'neg_half' channel permutation P to rows of W
    (per 64-row head block): y[d] = -x[d+16] (d<16), x[d-16] (16<=d<32), 0."""
    Wp = np.zeros_like(W)
    nheads = W.shape[0] // HD
    for h in range(nheads):
        o = h * HD
        Wp[o:o + HALF] = -W[o + HALF:o + D_ROPE]
        Wp[o + HALF:o + D_ROPE] = W[o:o + HALF]
    return Wp


def _build_nc():
    import concourse.tile as tile
    from concourse import bacc, mybir

    F32 = mybir.dt.float32
    F32R = mybir.dt.float32r
    AF = mybir.ActivationFunctionType

    nc = bacc.Bacc(name="mha")
    dram = {}
    for name, shape, dt in [
        ("x", (C, T), F32R), ("cc", (C, T), F32R),
        ("wqT", (C, CHG), F32R), ("wqpT", (C, CHG), F32R),
        ("wkT", (C, CHG), F32R), ("wkpT", (C, CHG), F32R),
        ("wvT", (C, CHG), F32R), ("woT", (CHG, C), F32R),
        ("bq", (1, CHG), F32R), ("bqp", (1, CHG), F32R),
        ("bk", (1, CHG), F32R), ("bkp", (1, CHG), F32R),
        ("bv", (1, CHG), F32R), ("bo", (1, C), F32R),
        ("Ct", (128, T), F32), ("St", (128, T), F32),
        ("onesT", (1, T), F32R), ("ones128", (128, 1), F32R),
    ]:
        dram[name] = nc.dram_tensor(name, shape, dt, kind="ExternalInput")
    out = nc.dram_tensor("out", (C, T), F32, kind="ExternalOutput")

    with tile.TileContext(nc) as tc:
        with tc.tile_pool(name="io", bufs=1) as io, \
             tc.tile_pool(name="wq", bufs=3) as wpool, \
             tc.tile_pool(name="qk", bufs=1) as qkpool, \
             tc.tile_pool(name="pp", bufs=3) as ppool, \
             tc.tile_pool(name="sc", bufs=2) as spool, \
             tc.tile_pool(name="ob", bufs=4) as opool, \
             tc.tile_pool(name="psq", bufs=1, space="PSUM") as psq, \
             tc.tile_pool(name="psqp", bufs=1, space="PSUM") as psqp, \
             tc.tile_pool(name="pss", bufs=2, space="PSUM") as pss, \
             tc.tile_pool(name="pspv", bufs=1, space="PSUM") as pspv:

            # ---------- resident loads ----------
            xt = io.tile([128, KT, T], F32R, tag="x")
            ct = io.tile([128, KT, T], F32R, tag="c")
            for k in range(KT):
                nc.sync.dma_start(xt[:, k], dram["x"][k * 128:(k + 1) * 128, :])
                nc.sync.dma_start(ct[:, k], dram["cc"][k * 128:(k + 1) * 128, :])
            Ctt = io.tile([128, T], F32, tag="Ct")
            Stt = io.tile([128, T], F32, tag="St")
            nc.sync.dma_start(Ctt[:], dram["Ct"][:])
            nc.sync.dma_start(Stt[:], dram["St"][:])
            ones_row = io.tile([1, T], F32R, tag="ones")
            nc.sync.dma_start(ones_row[:], dram["onesT"][:])
            ones_col = io.tile([128, 1], F32R, tag="ones_col")
            nc.sync.dma_start(ones_col[:], dram["ones128"][:])
            brow = {}
            for bn in ("bq", "bqp", "bk", "bkp", "bv", "bo"):
                brow[bn] = io.tile([1, dram[bn].shape[1]], F32R, tag=bn, name=bn)
                nc.sync.dma_start(brow[bn][:], dram[bn][:])

            # ---------- q/k projections fused with rope ----------
            # qr/kr: [128, MT, T] f32r, head-pair hp in sub-tile hp
            qr = qkpool.tile([128, MT, T], F32R, tag="qr")
            kr = qkpool.tile([128, MT, T], F32R, tag="kr")

            def proj_rope(wT, wpT, bn, bpn, res, src):
                for m in range(MT):
                    for j in range(NT):
                        tsl = slice(j * TC, (j + 1) * TC)
                        csl = slice(m * 128, (m + 1) * 128)
                        pq = psq.tile([128, TC], F32, tag="ps_q")
                        pp_ = psqp.tile([128, TC], F32, tag="ps_qp")
                        for k in range(KT):
                            wt = wpool.tile([128, 128], F32R, tag="w_qk")
                            nc.sync.dma_start(wt[:], wT[k * 128:(k + 1) * 128, csl])
                            nc.tensor.matmul(pq[:], wt[:], src[:, k, tsl],
                                             start=(k == 0), stop=False)
                        nc.tensor.matmul(pq[:], brow[bn][:, csl], ones_row[:, tsl],
                                         start=False, stop=True)
                        for k in range(KT):
                            wt = wpool.tile([128, 128], F32R, tag="w_qk")
                            nc.sync.dma_start(wt[:], wpT[k * 128:(k + 1) * 128, csl])
                            nc.tensor.matmul(pp_[:], wt[:], src[:, k, tsl],
                                             start=(k == 0), stop=False)
                        nc.tensor.matmul(pp_[:], brow[bpn][:, csl], ones_row[:, tsl],
                                         start=False, stop=True)
                        t1 = spool.tile([128, TC], F32, tag="rope1")
                        t2 = spool.tile([128, TC], F32, tag="rope2")
                        nc.vector.tensor_mul(t1[:], pq[:], Ctt[:, tsl])
                        nc.vector.tensor_mul(t2[:], pp_[:], Stt[:, tsl])
                        nc.vector.tensor_add(res[:, m, tsl], t1[:], t2[:])

            proj_rope(dram["wqT"], dram["wqpT"], "bq", "bqp", qr, xt)
            proj_rope(dram["wkT"], dram["wkpT"], "bk", "bkp", kr, ct)

            # ---------- v^T projection into head-padded layout ----------
            # vt[st]: [128 s, NH, HD+1] f32r, ones in column HD
            vts = []
            for st in range(ST):
                vt = qkpool.tile([128, NH, HD + 1], F32R, tag=f"vt{st}")
                pv_ = psq.tile([128, CHG], F32, tag="ps_q")
                ssl = slice(st * 128, (st + 1) * 128)
                for k in range(KT):
                    wt = wpool.tile([128, CHG], F32R, tag="w_v")
                    nc.sync.dma_start(wt[:], dram["wvT"][k * 128:(k + 1) * 128, :])
                    nc.tensor.matmul(pv_[:], ct[:, k, ssl], wt[:],
                                     start=(k == 0), stop=False)
                nc.tensor.matmul(pv_[:], ones_row[:, ssl], brow["bv"][:],
                                 start=False, stop=True)
                nc.vector.tensor_copy(
                    vt[:, :, 0:HD],
                    pv_[:].rearrange("p (h d) -> p h d", h=NH))
                nc.vector.tensor_copy(vt[:, :, HD],
                                      ones_col[:].to_broadcast([128, NH]))
                vts.append(vt)

            # ---------- attention ----------
            att = qkpool.tile([128, MT, T], F32R, tag="att")
            for hp in range(NPAIR):
                for j in range(NT):
                    tsl = slice(j * TC, (j + 1) * TC)
                    pvA = pspv.tile([HD + 1, TC], F32, tag="pvA")
                    pvB = pspv.tile([HD + 1, TC], F32, tag="pvB")
                    for st in range(ST):
                        ssl = slice(st * 128, (st + 1) * 128)
                        sA = pss.tile([128, TC], F32, tag="sA")
                        sB = pss.tile([128, TC], F32, tag="sB")
                        nc.tensor.matmul(sA[:], kr[0:64, hp, ssl], qr[0:64, hp, tsl],
                                         start=True, stop=True)
                        nc.tensor.matmul(sB[:], kr[64:128, hp, ssl], qr[64:128, hp, tsl],
                                         start=True, stop=True)
                        pA = ppool.tile([128, TC], F32R, tag="pA")
                        pB = ppool.tile([128, TC], F32R, tag="pB")
                        nc.scalar.activation(pA[:], sA[:], AF.Exp, scale=SCALE)
                        nc.scalar.activation(pB[:], sB[:], AF.Exp, scale=SCALE)
                        nc.tensor.matmul(pvA[:], vts[st][:, 2 * hp], pA[:],
                                         start=(st == 0), stop=(st == ST - 1))
                        nc.tensor.matmul(pvB[:], vts[st][:, 2 * hp + 1], pB[:],
                                         start=(st == 0), stop=(st == ST - 1))
                    for half, pv in ((0, pvA), (1, pvB)):
                        rec = spool.tile([1, TC], F32, tag="rec")
                        nc.vector.reciprocal(rec[:], pv[HD:HD + 1, :])
                        bc = spool.tile([HD, TC], F32, tag="bc")
                        nc.gpsimd.partition_broadcast(bc[:], rec[:])
                        nc.vector.tensor_mul(att[half * HD:(half + 1) * HD, hp, tsl],
                                             pv[0:HD, :], bc[:])

            # ---------- partial O projection ----------
            for m in range(OMT):
                osl = slice(m * 128, (m + 1) * 128)
                wo_t = wpool.tile([128, MT, 128], F32R, tag="w_o")
                for k in range(MT):
                    nc.sync.dma_start(wo_t[:, k], dram["woT"][k * 128:(k + 1) * 128, osl])
                for j in range(NT):
                    tsl = slice(j * TC, (j + 1) * TC)
                    po = psqp.tile([128, TC], F32, tag="ps_qp")
                    for k in range(MT):
                        nc.tensor.matmul(po[:], wo_t[:, k], att[:, k, tsl],
                                         start=(k == 0), stop=False)
                    nc.tensor.matmul(po[:], brow["bo"][:, osl], ones_row[:, tsl],
                                     start=False, stop=True)
                    ot = opool.tile([128, TC], F32, tag="o_sb")
                    nc.vector.tensor_copy(ot[:], po[:])
                    nc.sync.dma_start(out[osl, tsl], ot[:])
    nc.finalize()
    return nc


def _get_runner():
    """Build the Bass program once and return a cached callable
    (in_maps) -> list of per-core {out: np.ndarray}."""
    if "runner" in _cache:
        return _cache["runner"]
    from concourse.bass_utils import run_bass_kernel_spmd

    nc = _build_nc()

    def runner(in_maps):
        t0 = time.perf_counter()
        res = run_bass_kernel_spmd(nc, in_maps, core_ids=list(range(NCORES)))
        t1 = time.perf_counter()
        runner.last_wall_s = t1 - t0
        return res.results

    _cache["runner"] = runner
    return runner


def _prep_in_maps(x, c, Wq, bq, Wk, bk, Wv, bv, Wo, bo):
    Ct, St = _cs_tiles()
    onesT = np.ones((1, T), dtype=np.float32)
    x = np.asarray(x, dtype=np.float32)
    c = np.asarray(c, dtype=np.float32)
    in_maps = []
    for b in range(B):
        for g in range(GROUPS):
            gsl = slice(g * CHG, (g + 1) * CHG)
            Wq_g, Wk_g, Wv_g = Wq[gsl], Wk[gsl], Wv[gsl]
            bq_g, bk_g, bv_g = bq[gsl], bk[gsl], bv[gsl]
            in_maps.append({
                "x": np.ascontiguousarray(x[b]),
                "cc": np.ascontiguousarray(c[b]),
                "wqT": np.ascontiguousarray(Wq_g.T),
                "wqpT": np.ascontiguousarray(_perm_rows(Wq_g).T),
                "wkT": np.ascontiguousarray(Wk_g.T),
                "wkpT": np.ascontiguousarray(_perm_rows(Wk_g).T),
                "wvT": np.ascontiguousarray(Wv_g.T),
                "woT": np.ascontiguousarray(Wo[:, gsl].T),
                "bq": bq_g[None, :].astype(np.float32),
                "bqp": _perm_rows(bq_g[:, None])[:, 0][None, :].astype(np.float32),
                "bk": bk_g[None, :].astype(np.float32),
                "bkp": _perm_rows(bk_g[:, None])[:, 0][None, :].astype(np.float32),
                "bv": bv_g[None, :].astype(np.float32),
                "bo": (bo[None, :] if g == 0
                       else np.zeros((1, C))).astype(np.float32),
                "Ct": Ct, "St": St, "onesT": onesT,
                "ones128": np.ones((128, 1), dtype=np.float32),
            })
    return in_maps


def kernel(x, c, attn_mask, Wq, bq, Wk, bk, Wv, bv, Wo, bo):
    # attn_mask is all-ones per the problem spec; the where() in the
    # reference is a no-op, so it is not applied on-device.
    runner = _get_runner()
    in_maps = _prep_in_maps(np.asarray(x), np.asarray(c),
                            np.asarray(Wq), np.asarray(bq),
                            np.asarray(Wk), np.asarray(bk),
                            np.asarray(Wv), np.asarray(bv),
                            np.asarray(Wo), np.asarray(bo))
    results = runner(in_maps)
    out = np.empty((B, C, T), dtype=np.float32)
    for b in range(B):
        out[b] = results[2 * b]["out"] + results[2 * b + 1]["out"]
    return out


# revision 6
# speedup vs baseline: 11.3923x; 11.3923x over previous
"""Trainium2 Bass kernel for nn_MultiHeadAttention (B=4, C=1024, T=1024, H=16).

Sharding: 8 cores = (batch b in 0..3) x (head-group g in 0..1), 8 heads per
group. Each core computes q/k/v projections for its group's 512 channels,
rope, attention, and a partial O-projection Wo[:, group] @ att. The host sums
the two partials per batch (bias bo is supplied only to g=0 cores).

Layout strategy (everything stays in natural [channel, t] / [s, t] layouts,
zero on-device transposes; weights are pre-transposed on the host):
  - scores computed transposed: scoresT[s, t] = k[d, s].T @ q[d, t] per head,
    two heads packed per 128-partition tile via PE row-tiling.
  - softmax: no max-subtraction (scores are O(5), mask is all-ones so it is
    skipped), exp on ScalarE straight from PSUM, denominator = extra ones
    column in v^T so the PV matmul emits it for free; normalization via
    reciprocal + gpsimd partition_broadcast.
  - rope folded into the projections: q_rope = C.*q + S.*(P q) where P is a
    fixed signed channel permutation; (P Wq) is computed on the host, the
    elementwise part runs on VectorE reading both projection PSUMs directly.
  - all matmuls run in float32r (full PE rate, ~TF32 precision, fp32 storage).
  - biases are added via K=1 rank-1 matmuls (bias row x ones row) inside each
    accumulation group.
"""
import sys
import time

sys.path.insert(0, '/opt/trn_rl_repo')

import numpy as np

B = 4
C = 1024
T = 1024
H = 16
HD = C // H            # 64
D_ROPE = HD // 2       # 32
HALF = D_ROPE // 2     # 16
GROUPS = 2
NCORES = 8
NH = H // GROUPS       # 8 heads per group
CHG = NH * HD          # 512 channels per group
KT = C // 128          # 8 input-channel k-tiles
ST = T // 128          # 8 s-tiles
TC = 512
NT = T // TC           # 2 t-chunks
MT = CHG // 128        # 4 projection m-tiles per group
OMT = C // 128         # 8 output m-tiles
NPAIR = NH // 2        # 4 head-pairs (2 heads packed per 128-tile)
SCALE = 0.125          # 1/sqrt(HD)

_cache = {}


def _rope_tables():
    theta = 1.0 / (10000.0 ** (np.arange(HALF, dtype=np.float64) * 2.0 / D_ROPE))
    ang = np.arange(T, dtype=np.float64)[:, None] * theta[None, :]   # [T, HALF]
    cos = np.concatenate([np.cos(ang), np.cos(ang)], axis=1)         # [T, D_ROPE]
    sin = np.concatenate([np.sin(ang), np.sin(ang)], axis=1)
    return cos.astype(np.float32), sin.astype(np.float32)


def _cs_tiles():
    """C,S tables in [128 ch, T] layout; the 2-head (64-row) pattern repeats,
    so one 128-row tile serves every projection m-tile."""
    cos, sin = _rope_tables()
    Ct = np.ones((128, T), dtype=np.float32)
    St = np.zeros((128, T), dtype=np.float32)
    for h in range(2):
        o = h * HD
        Ct[o:o + D_ROPE, :] = cos.T
        St[o:o + D_ROPE, :] = sin.T
    return Ct, St


def _perm_rows(W):
    """Apply the rope 'neg_half' channel permutation P to rows of W
    (per 64-row head block): y[d] = -x[d+16] (d<16), x[d-16] (16<=d<32), 0."""
    Wp = np.zeros_like(W)
    nheads = W.shape[0] // HD
    for h in range(nheads):
        o = h * HD
        Wp[o:o + HALF] = -W[o + HALF:o + D_ROPE]
        Wp[o + HALF:o + D_ROPE] = W[o:o + HALF]
    return Wp


def _build_nc():
    import concourse.tile as tile
    from concourse import bacc, mybir

    F32 = mybir.dt.float32
    F32R = mybir.dt.float32r
    AF = mybir.ActivationFunctionType

    nc = bacc.Bacc(name="mha")
    dram = {}
    for name, shape, dt in [
        ("x", (C, T), F32R), ("cc", (C, T), F32R),
        ("wqT", (C, CHG), F32R), ("wqpT", (C, CHG), F32R),
        ("wkT", (C, CHG), F32R), ("wkpT", (C, CHG), F32R),
        ("wvT", (C, CHG), F32R), ("woT", (CHG, C), F32R),
        ("bq", (1, CHG), F32R), ("bqp", (1, CHG), F32R),
        ("bk", (1, CHG), F32R), ("bkp", (1, CHG), F32R),
        ("bv", (1, CHG), F32R), ("bo", (1, C), F32R),
        ("Ct", (128, T), F32), ("St", (128, T), F32),
        ("onesT", (1, T), F32R), ("ones128", (128, 1), F32R),
    ]:
        dram[name] = nc.dram_tensor(name, shape, dt, kind="ExternalInput")
    out = nc.dram_tensor("out", (C, T), F32, kind="ExternalOutput")

    with tile.TileContext(nc) as tc:
        with tc.tile_pool(name="io", bufs=1) as io, \
             tc.tile_pool(name="wq", bufs=3) as wpool, \
             tc.tile_pool(name="qk", bufs=1) as qkpool, \
             tc.tile_pool(name="pp", bufs=3) as ppool, \
             tc.tile_pool(name="sc", bufs=2) as spool, \
             tc.tile_pool(name="ob", bufs=4) as opool, \
             tc.tile_pool(name="psq", bufs=1, space="PSUM") as psq, \
             tc.tile_pool(name="psqp", bufs=1, space="PSUM") as psqp, \
             tc.tile_pool(name="pss", bufs=2, space="PSUM") as pss, \
             tc.tile_pool(name="pspv", bufs=1, space="PSUM") as pspv:

            # ---------- resident loads ----------
            xt = io.tile([128, KT, T], F32R, tag="x")
            ct = io.tile([128, KT, T], F32R, tag="c")
            for k in range(KT):
                nc.sync.dma_start(xt[:, k], dram["x"][k * 128:(k + 1) * 128, :])
                nc.sync.dma_start(ct[:, k], dram["cc"][k * 128:(k + 1) * 128, :])
            Ctt = io.tile([128, T], F32, tag="Ct")
            Stt = io.tile([128, T], F32, tag="St")
            nc.sync.dma_start(Ctt[:], dram["Ct"][:])
            nc.sync.dma_start(Stt[:], dram["St"][:])
            ones_row = io.tile([1, T], F32R, tag="ones")
            nc.sync.dma_start(ones_row[:], dram["onesT"][:])
            ones_col = io.tile([128, 1], F32R, tag="ones_col")
            nc.sync.dma_start(ones_col[:], dram["ones128"][:])
            brow = {}
            for bn in ("bq", "bqp", "bk", "bkp", "bv", "bo"):
                brow[bn] = io.tile([1, dram[bn].shape[1]], F32R, tag=bn, name=bn)
                nc.sync.dma_start(brow[bn][:], dram[bn][:])

            # ---------- q/k projections fused with rope ----------
            # qr/kr: [128, MT, T] f32r, head-pair hp in sub-tile hp
            qr = qkpool.tile([128, MT, T], F32R, tag="qr")
            kr = qkpool.tile([128, MT, T], F32R, tag="kr")

            def proj_rope(wT, wpT, bn, bpn, res, src):
                for m in range(MT):
                    for j in range(NT):
                        tsl = slice(j * TC, (j + 1) * TC)
                        csl = slice(m * 128, (m + 1) * 128)
                        pq = psq.tile([128, TC], F32, tag="ps_q")
                        pp_ = psqp.tile([128, TC], F32, tag="ps_qp")
                        for k in range(KT):
                            wt = wpool.tile([128, 128], F32R, tag="w_qk")
                            nc.sync.dma_start(wt[:], wT[k * 128:(k + 1) * 128, csl])
                            nc.tensor.matmul(pq[:], wt[:], src[:, k, tsl],
                                             start=(k == 0), stop=False)
                        nc.tensor.matmul(pq[:], brow[bn][:, csl], ones_row[:, tsl],
                                         start=False, stop=True)
                        for k in range(KT):
                            wt = wpool.tile([128, 128], F32R, tag="w_qk")
                            nc.sync.dma_start(wt[:], wpT[k * 128:(k + 1) * 128, csl])
                            nc.tensor.matmul(pp_[:], wt[:], src[:, k, tsl],
                                             start=(k == 0), stop=False)
                        nc.tensor.matmul(pp_[:], brow[bpn][:, csl], ones_row[:, tsl],
                                         start=False, stop=True)
                        t1 = spool.tile([128, TC], F32, tag="rope1")
                        t2 = spool.tile([128, TC], F32, tag="rope2")
                        nc.vector.tensor_mul(t1[:], pq[:], Ctt[:, tsl])
                        nc.vector.tensor_mul(t2[:], pp_[:], Stt[:, tsl])
                        nc.vector.tensor_add(res[:, m, tsl], t1[:], t2[:])

            proj_rope(dram["wqT"], dram["wqpT"], "bq", "bqp", qr, xt)
            proj_rope(dram["wkT"], dram["wkpT"], "bk", "bkp", kr, ct)

            # ---------- v^T projection into head-padded layout ----------
            # vt[st]: [128 s, NH, HD+1] f32r, ones in column HD
            vts = []
            for st in range(ST):
                vt = qkpool.tile([128, NH, HD + 1], F32R, tag=f"vt{st}")
                pv_ = psq.tile([128, CHG], F32, tag="ps_q")
                ssl = slice(st * 128, (st + 1) * 128)
                for k in range(KT):
                    wt = wpool.tile([128, CHG], F32R, tag="w_v")
                    nc.sync.dma_start(wt[:], dram["wvT"][k * 128:(k + 1) * 128, :])
                    nc.tensor.matmul(pv_[:], ct[:, k, ssl], wt[:],
                                     start=(k == 0), stop=False)
                nc.tensor.matmul(pv_[:], ones_row[:, ssl], brow["bv"][:],
                                 start=False, stop=True)
                nc.vector.tensor_copy(
                    vt[:, :, 0:HD],
                    pv_[:].rearrange("p (h d) -> p h d", h=NH))
                nc.vector.tensor_copy(vt[:, :, HD],
                                      ones_col[:].to_broadcast([128, NH]))
                vts.append(vt)

            # ---------- attention ----------
            att = qkpool.tile([128, MT, T], F32R, tag="att")
            for hp in range(NPAIR):
                for j in range(NT):
                    tsl = slice(j * TC, (j + 1) * TC)
                    pvA = pspv.tile([HD + 1, TC], F32, tag="pvA")
                    pvB = pspv.tile([HD + 1, TC], F32, tag="pvB")
                    for st in range(ST):
                        ssl = slice(st * 128, (st + 1) * 128)
                        sA = pss.tile([128, TC], F32, tag="sA")
                        sB = pss.tile([128, TC], F32, tag="sB")
                        nc.tensor.matmul(sA[:], kr[0:64, hp, ssl], qr[0:64, hp, tsl],
                                         start=True, stop=True)
                        nc.tensor.matmul(sB[:], kr[64:128, hp, ssl], qr[64:128, hp, tsl],
                                         start=True, stop=True)
                        pA = ppool.tile([128, TC], F32R, tag="pA")
                        pB = ppool.tile([128, TC], F32R, tag="pB")
                        nc.scalar.activation(pA[:], sA[:], AF.Exp, scale=SCALE)
                        nc.scalar.activation(pB[:], sB[:], AF.Exp, scale=SCALE)
                        nc.tensor.matmul(pvA[:], vts[st][:, 2 * hp], pA[:],
                                         start=(st == 0), stop=(st == ST - 1))
                        nc.tensor.matmul(pvB[:], vts[st][:, 2 * hp + 1], pB[:],
                                         start=(st == 0), stop=(st == ST - 1))
                    for half, pv in ((0, pvA), (1, pvB)):
                        rec = spool.tile([1, TC], F32, tag="rec")
                        nc.vector.reciprocal(rec[:], pv[HD:HD + 1, :])
                        bc = spool.tile([HD, TC], F32, tag="bc")
                        nc.gpsimd.partition_broadcast(bc[:], rec[:])
                        nc.vector.tensor_mul(att[half * HD:(half + 1) * HD, hp, tsl],
                                             pv[0:HD, :], bc[:])

            # ---------- partial O projection ----------
            for m in range(OMT):
                osl = slice(m * 128, (m + 1) * 128)
                wo_t = wpool.tile([128, MT, 128], F32R, tag="w_o")
                for k in range(MT):
                    nc.sync.dma_start(wo_t[:, k], dram["woT"][k * 128:(k + 1) * 128, osl])
                for j in range(NT):
                    tsl = slice(j * TC, (j + 1) * TC)
                    po = psqp.tile([128, TC], F32, tag="ps_qp")
                    for k in range(MT):
                        nc.tensor.matmul(po[:], wo_t[:, k], att[:, k, tsl],
                                         start=(k == 0), stop=False)
                    nc.tensor.matmul(po[:], brow["bo"][:, osl], ones_row[:, tsl],
                                     start=False, stop=True)
                    ot = opool.tile([128, TC], F32, tag="o_sb")
                    nc.vector.tensor_copy(ot[:], po[:])
                    nc.sync.dma_start(out[osl, tsl], ot[:])
    nc.finalize()
    return nc


def _get_runner():
    """Build the Bass program once, wrap it in a cached jitted shard_map
    callable (mirrors bass2jax.run_bass_via_pjrt)."""
    if "runner" in _cache:
        return _cache["runner"]

    import jax
    from jax.sharding import Mesh, PartitionSpec, NamedSharding
    from jax.experimental.shard_map import shard_map
    from concourse import bass2jax, mybir

    bass2jax.install_neuronx_cc_hook()
    nc = _build_nc()

    partition_name = (nc.partition_id_tensor.name
                      if nc.partition_id_tensor else None)
    in_names, out_names, out_avals, zero_shapes = [], [], [], []
    for alloc in nc.m.functions[0].allocations:
        if not isinstance(alloc, mybir.MemoryLocationSet):
            continue
        name = alloc.memorylocations[0].name
        if alloc.kind == "ExternalInput":
            if name != partition_name:
                in_names.append(name)
        elif alloc.kind == "ExternalOutput":
            shape = tuple(alloc.tensor_shape)
            dtype = mybir.dt.np(alloc.dtype)
            out_names.append(name)
            out_avals.append(jax.core.ShapedArray(shape, dtype))
            zero_shapes.append((shape, dtype))
    n_params = len(in_names)
    all_names = list(in_names) + list(out_names)
    if partition_name is not None:
        all_names.append(partition_name)
    donate = tuple(range(n_params, n_params + len(out_names)))

    def _body(*args):
        operands = list(args)
        if partition_name is not None:
            operands.append(bass2jax.partition_id_tensor())
        outs = bass2jax._bass_exec_p.bind(
            *operands,
            out_avals=tuple(out_avals),
            in_names=tuple(all_names),
            out_names=tuple(out_names),
            lowering_input_output_aliases=(),
            sim_require_finite=True,
            sim_require_nnan=True,
            nc=nc,
        )
        return tuple(outs)

    devices = jax.devices()[:NCORES]
    mesh = Mesh(np.asarray(devices), ("core",))
    n_out = len(out_names)
    in_specs = (PartitionSpec("core"),) * (n_params + n_out)
    out_specs = (PartitionSpec("core"),) * n_out
    sharded = jax.jit(
        shard_map(_body, mesh=mesh, in_specs=in_specs, out_specs=out_specs,
                  check_rep=False),
        donate_argnums=donate, keep_unused=True)
    core_sharding = NamedSharding(mesh, PartitionSpec("core"))

    class Runner:
        def device_put(self, in_maps):
            """Concat per-core inputs and place them on the devices."""
            concat = [
                np.concatenate([m[name] for m in in_maps], axis=0)
                for name in in_names
            ]
            return [jax.device_put(a, core_sharding) for a in concat]

        def zeros(self):
            return [np.zeros((NCORES * s[0], *s[1:]), d) for s, d in zero_shapes]

        def execute(self, placed):
            out = sharded(*placed, *self.zeros())
            jax.block_until_ready(out)
            return out

        def __call__(self, in_maps):
            t0 = time.perf_counter()
            placed = self.device_put(in_maps)
            t1 = time.perf_counter()
            out_arrs = self.execute(placed)
            t2 = time.perf_counter()
            self.last_transfer_s = t1 - t0
            self.last_exec_s = t2 - t1
            self.last_wall_s = t2 - t0
            return [
                {name: np.asarray(out_arrs[i]).reshape(NCORES, *out_avals[i].shape)[c]
                 for i, name in enumerate(out_names)}
                for c in range(NCORES)
            ]

    runner = Runner()
    _cache["runner"] = runner
    return runner


def _prep_in_maps(x, c, Wq, bq, Wk, bk, Wv, bv, Wo, bo):
    Ct, St = _cs_tiles()
    onesT = np.ones((1, T), dtype=np.float32)
    x = np.asarray(x, dtype=np.float32)
    c = np.asarray(c, dtype=np.float32)
    in_maps = []
    for b in range(B):
        for g in range(GROUPS):
            gsl = slice(g * CHG, (g + 1) * CHG)
            Wq_g, Wk_g, Wv_g = Wq[gsl], Wk[gsl], Wv[gsl]
            bq_g, bk_g, bv_g = bq[gsl], bk[gsl], bv[gsl]
            in_maps.append({
                "x": np.ascontiguousarray(x[b]),
                "cc": np.ascontiguousarray(c[b]),
                "wqT": np.ascontiguousarray(Wq_g.T),
                "wqpT": np.ascontiguousarray(_perm_rows(Wq_g).T),
                "wkT": np.ascontiguousarray(Wk_g.T),
                "wkpT": np.ascontiguousarray(_perm_rows(Wk_g).T),
                "wvT": np.ascontiguousarray(Wv_g.T),
                "woT": np.ascontiguousarray(Wo[:, gsl].T),
                "bq": bq_g[None, :].astype(np.float32),
                "bqp": _perm_rows(bq_g[:, None])[:, 0][None, :].astype(np.float32),
                "bk": bk_g[None, :].astype(np.float32),
                "bkp": _perm_rows(bk_g[:, None])[:, 0][None, :].astype(np.float32),
                "bv": bv_g[None, :].astype(np.float32),
                "bo": (bo[None, :] if g == 0
                       else np.zeros((1, C))).astype(np.float32),
                "Ct": Ct, "St": St, "onesT": onesT,
                "ones128": np.ones((128, 1), dtype=np.float32),
            })
    return in_maps


def kernel(x, c, attn_mask, Wq, bq, Wk, bk, Wv, bv, Wo, bo):
    # attn_mask is all-ones per the problem spec; the where() in the
    # reference is a no-op, so it is not applied on-device.
    runner = _get_runner()
    in_maps = _prep_in_maps(np.asarray(x), np.asarray(c),
                            np.asarray(Wq), np.asarray(bq),
                            np.asarray(Wk), np.asarray(bk),
                            np.asarray(Wv), np.asarray(bv),
                            np.asarray(Wo), np.asarray(bo))
    results = runner(in_maps)
    out = np.empty((B, C, T), dtype=np.float32)
    for b in range(B):
        out[b] = results[2 * b]["out"] + results[2 * b + 1]["out"]
    return out
